# revision 15
# baseline (speedup 1.0000x reference)
"""Trainium2 Bass kernel for nn_CoKT — v2 (chain-optimized rewrite).

Design (per core: 128 own tokens, 768 inter seqs, weights replicated):
- The intra-GRU 64-step serial chain paces the kernel (measured 4.7us/step in
  v1 = 300us).  v2 shortens it: contiguous ping-pong h tiles, z-path split
  (h' = (h - zc*h) + zc*n) so only 2 ops trail tanh, post-sigmoid ops on the
  otherwise-idle GpSimd engine, per-step history stored only for own batches.
- Inter GRU runs in 512-wide blocks with per-ci PSUM banks (6 banks), halving
  ACT/PE instruction counts per column vs 256-wide tiles; h-updates on DVE in
  4x mode (was GpSimd at ~1.1us/op); xn input-projection pipelined through a
  dedicated 1-bank chunk queue with DVE evacuation.
- DMA priority order + split xinter so the chain starts ~3us in (was 34us).
- Phase 3 reordered: PE-heavy projections emitted first to cover the exp
  table load, independent softmax chains interleaved.
"""
import sys
if "/opt/trn_rl_repo" not in sys.path:
    sys.path.insert(0, "/opt/trn_rl_repo")

import numpy as np
import ml_dtypes

import concourse.bacc as bacc
import concourse.mybir as mybir
import concourse.tile as tile
from concourse.tile import add_dep_helper
from concourse.bass_utils import run_bass_kernel_spmd

F32 = mybir.dt.float32
BF16 = mybir.dt.bfloat16
F8 = mybir.dt.float8e4
PM_DR = mybir.MatmulPerfMode.DoubleRow
AF = mybir.ActivationFunctionType
ALU = mybir.AluOpType
AX = mybir.AxisListType

B, S, R, L, D, H = 16, 64, 6, 24, 128, 256
NCORES = 8
BPC = B // NCORES            # 2 batches per core
NTOK = S * BPC               # 128 own tokens
NSEQ = NTOK * R              # 768 inter sequences per core
NBW = 512                    # inter block width (1 PSUM bank per ci)
XCW = 256                    # xn chunk width
BIG = 30000.0
NST = NSEQ // 128            # 6 seq-tiles of 128 in the attention phase

bfc = lambda x: np.ascontiguousarray(np.asarray(x, np.float32).astype(ml_dtypes.bfloat16))
f32c = lambda x: np.ascontiguousarray(np.asarray(x, np.float32))

_BLOB_NAMES = [
    ("iqw0", 256), ("iqw1", 256), ("ikw0", 256), ("ikw1", 256),
    ("ivw0", 256), ("ivw1", 256), ("ivwx", 256), ("avw0", 256), ("avw1", 256),
    ("AiT0", 256), ("AiT1", 256), ("AaT0", 256), ("AaT1", 256),
    ("LhT0", 256), ("LhT1", 256),
    ("iqwx", 256), ("ikwx", 256), ("aqw", 256), ("akw", 256), ("LxT", 256),
    ("id128", 128), ("Pq", NSEQ), ("Pi", NSEQ), ("cmask", S),
]
_BLOB_OFF = {}
_off = 0
for _nm, _w in _BLOB_NAMES:
    _BLOB_OFF[_nm] = (_off, _w)
    _off += _w
BLOBW = _off
_ROWS127 = {"iqwx", "ikwx", "aqw", "akw", "LxT"}

_BLOBC_NAMES = ["iqb", "ikb", "ivb", "avwx", "avb", "btot"]
_BLOBD_NAMES = ["b_r", "nb_z", "b_in", "b_hn", "aqb", "akb"]


def _blocks_of(w, step=NBW):
    out = []
    o = 0
    while o < w:
        out.append((o, min(step, w - o)))
        o += step
    return out


# ----------------------------------------------------------------------------
# device program
# ----------------------------------------------------------------------------

def _coloc(insts):
    first = insts[0]
    for x in insts[1:]:
        add_dep_helper(x.ins, first.ins, sync=True, reason="psum coloc order")


def _after(consumer, last_mm):
    """Reader of a PSUM bank must wait until the PE is done with the bank."""
    add_dep_helper(consumer.ins, last_mm.ins, sync=True, reason="bank read-after-all-mm")


def _emit(nc, tc, di, d_out, W, OFF, MINACT):
    import contextlib
    ctx = contextlib.ExitStack()
    TOTAL = OFF[-1] + W[-1]
    NCH = (TOTAL + XCW - 1) // XCW
    with ctx:
        singles = ctx.enter_context(tc.tile_pool(name="singles", bufs=1))
        sb2 = ctx.enter_context(tc.tile_pool(name="work2", bufs=2))
        sb3 = ctx.enter_context(tc.tile_pool(name="work3", bufs=3))

        def load(name, cols=None):
            d = di[name]
            t = singles.tile(list(d.shape), d.dtype, tag=name)
            if cols is None:
                nc.sync.dma_start(out=t, in_=d.ap())
            else:
                a, b_ = cols
                nc.sync.dma_start(out=t[:, a:b_], in_=d.ap()[:, a:b_])
            return t

        # DMA priority order: intra-scan inputs first so the chain starts early
        wihT = load("wihT")
        xintra = load("xintra")
        blobD = load("blobD")
        bhnT = load("bhnT")
        ind2 = load("ind2")
        whhT = [load("whh0T"), load("whh1T")]
        whh8 = load("whh8")
        id128e = load("id128e")
        XSPLIT = min(1024, TOTAL)
        xin = load("xinter", cols=(0, XSPLIT))
        ind_all = load("indr")
        nc.sync.dma_start(out=xin[:, XSPLIT:TOTAL], in_=di["xinter"].ap()[:, XSPLIT:TOTAL])
        rT = load("rT")
        xlast = load("xlast")
        blobB = singles.tile([128, BLOBW], BF16, tag="blobB")
        nc.sync.dma_start(out=blobB, in_=di["blobB"].ap())
        blobC = singles.tile([1, 256 * len(_BLOBC_NAMES)], BF16, tag="blobC")
        nc.sync.dma_start(out=blobC, in_=di["blobC"].ap())

        bD = {nm: blobD[:, 2 * i:2 * i + 2] for i, nm in enumerate(_BLOBD_NAMES)}
        b_r, nb_z, b_in, b_hn = bD["b_r"], bD["nb_z"], bD["b_in"], bD["b_hn"]
        aqb, akb = bD["aqb"], bD["akb"]
        W_ = {}
        for nm, (o_, w_) in _BLOB_OFF.items():
            rows = 127 if nm in _ROWS127 else (S if nm == "cmask" else 128)
            W_[nm] = blobB[0:rows, o_:o_ + w_]
        for i, nm in enumerate(_BLOBC_NAMES):
            W_[nm] = blobC[:, 256 * i:256 * (i + 1)]
        W_["id128"] = id128e

        ones = singles.tile([1, 128], BF16, tag="ones")
        nc.vector.memset(ones, 1.0)
        jsrc = singles.tile([128, 512], BF16, tag="jsrc")
        nc.vector.memset(jsrc, 0.5)

        xn_all = singles.tile([128, 2, TOTAL], BF16, tag="xn_all")
        xn_intra = singles.tile([128, 2, S, B], BF16, tag="xn_intra")
        xw4 = singles.tile([128, 4, B, S], BF16, tag="xw4")
        hT_own = singles.tile([128, 2, BPC, S], BF16, tag="hT_own")
        hpp = singles.tile([128, 2, 2, B], BF16, tag="hpp")
        nc.vector.memset(hpp, 0.0)
        h_inter = singles.tile([128, 2, NSEQ], BF16, tag="h_inter")
        nc.vector.memset(h_inter, 0.0)
        h8 = singles.tile([128, 2, NSEQ], F8, tag="h8")
        nc.vector.memset(h8, 0.0)

        # PSUM: 6 banks inter gates + 1 bank xn pipeline + 1 bank intra = 8
        gate_ps = tc.tile_pool(name="pg", bufs=1, space="PSUM")
        pg = gate_ps.__enter__()
        xn_ps = tc.tile_pool(name="pxn", bufs=1, space="PSUM")
        pxn = xn_ps.__enter__()
        ia_ps = tc.tile_pool(name="pia", bufs=1, space="PSUM")
        pia = ia_ps.__enter__()

        # ---------------- phase 1: intra input projections, by s-chunk -------
        def xprep_chunk(c):
            s0 = 16 * c
            xv = xintra[:, :, s0:s0 + 16]             # [128, B, 16] (b outer)
            for g in range(4):
                tag = ("rr", "zz")[g // 2]
                gt = pg.tile([128, 2, NBW], F32, tag=tag)
                px = gt[:, g % 2, 0:B * 16]
                m = nc.tensor.matmul(px, wihT[:, g * 128:(g + 1) * 128],
                                     xv, start=True, stop=True)
                bias = b_r[:, g:g + 1] if g < 2 else nb_z[:, g - 2:g - 1]
                ev = nc.vector.tensor_scalar_add(
                    xw4[:, g, :, s0:s0 + 16],
                    px.rearrange("p (b s) -> p b s", b=B), bias)
                _after(ev, m)
            nt = pg.tile([128, 2, NBW], F32, tag="nn")
            xv_sb = xintra[:, :, s0:s0 + 16].rearrange("p b s -> p s b")
            for ci in range(2):
                px = nt[:, ci, 0:B * 16]
                m = nc.tensor.matmul(px, wihT[:, 512 + ci * 128:640 + ci * 128],
                                     xv_sb, start=True, stop=True)
                ev = nc.vector.tensor_scalar_add(
                    xn_intra[:, ci, s0:s0 + 16, :],
                    px.rearrange("p (s b) -> p s b", s=16), b_in[:, ci:ci + 1])
                _after(ev, m)


        # pending-work queues for the round template
        from collections import deque
        act_pend = deque()    # inter sigmoid pieces (fill the chain's ACT gaps)
        dve_pend = deque()    # inter stt/u/tanh/d/f/h combined pieces
        pool_pend = deque()   # xn evacuations (run in Pool's idle tail)
        kv_ready = []
        kv_pieces = deque()

        # ---------------- intra GRU step (the critical chain) ----------------
        ictx = {}

        def intra_mm_sig(s):
            hprev = hpp[:, (s + 1) % 2]
            I = pia.tile([128, 32, B], F32, tag="ia")
            ictx["I"] = I
            ia = I[:, 0:4, :]
            ib = I[:, 4:6, :]
            id_mm = nc.tensor.matmul(ia.rearrange("p g b -> p (g b)"), id128e,
                                     xw4[:, :, :, s].rearrange("p g b -> p (g b)"),
                                     start=True, stop=False)
            insts = [id_mm]
            for g in range(4):
                sl = slice(g * 128, (g + 1) * 128)
                insts.append(nc.tensor.matmul(ia[:, g, :], whhT[0][:, sl],
                                              hprev[:, 0, :], start=False, stop=False))
                nc.tensor.matmul(ia[:, g, :], whhT[1][:, sl], hprev[:, 1, :],
                                 start=False, stop=False)
            for ci in range(2):
                sl = slice(512 + ci * 128, 512 + (ci + 1) * 128)
                nc.tensor.matmul(ib[:, ci, :], whhT[0][:, sl], hprev[:, 0, :],
                                 start=False, stop=False)
                nc.tensor.matmul(ib[:, ci, :], whhT[1][:, sl], hprev[:, 1, :],
                                 start=False, stop=False)
            fold = nc.tensor.matmul(ib.rearrange("p c b -> p (c b)"), bhnT,
                                    ind2, start=False, stop=True)
            _coloc(insts)
            rz4 = sb2.tile([128, 4, B], BF16, tag="irz4")
            sig = nc.scalar.activation(rz4, ia, AF.Sigmoid)
            _after(sig, fold)
            ictx.update(hprev=hprev, ib=ib, fold=fold, rz4=rz4)

        def intra_t1_u(s):
            t1 = sb2.tile([128, 2, B], BF16, tag="it1")
            u = sb2.tile([128, 2, B], BF16, tag="iu")
            _after(nc.vector.tensor_mul(t1, ictx["ib"], ictx["rz4"][:, 0:2, :]),
                   ictx["fold"])
            nc.vector.tensor_add(u, t1, xn_intra[:, :, s, :])
            ictx["u"] = u

        def intra_tail(s):
            hprev, rz4 = ictx["hprev"], ictx["rz4"]
            hnew = hpp[:, s % 2]
            zc = rz4[:, 2:4, :]
            n_sb = sb2.tile([128, 2, B], BF16, tag="in")
            p1a = sb2.tile([128, 2, B], BF16, tag="ip1a")
            p1 = sb2.tile([128, 2, B], BF16, tag="ip1")
            q = sb2.tile([128, 2, B], BF16, tag="iq")
            ictx["tanh"] = nc.scalar.activation(n_sb, ictx["u"], AF.Tanh)
            # off-chain z-path on Pool (runs while ACT does tanh)
            nc.gpsimd.tensor_mul(p1a, zc, hprev)
            nc.gpsimd.tensor_sub(p1, hprev, p1a)
            # chain tail on Pool
            nc.gpsimd.tensor_mul(q, zc, n_sb)
            nc.gpsimd.tensor_add(hnew, p1, q)
            # own-batch history (off-chain)
            nc.gpsimd.tensor_copy(hT_own[:, :, :, s], hnew[:, :, 0:BPC])

        # ---------------- inter GRU block: mm pieces + deferred follow-ups ---
        def make_block(bi, t, o, w):
            frz = (o + w) > MINACT[t]
            cols = slice(OFF[t] + o, OFF[t] + o + w)
            bctx = {}

            def gate_ci(Gt, goff, ci, extra_ones, out_sb, bias):
                sl = slice(goff + ci * 128, goff + (ci + 1) * 128)
                nc.tensor.matmul(Gt[:, ci, :w], wihT[:, sl], xin[:, cols],
                                 start=True, stop=False)
                last = nc.tensor.matmul(Gt[:, ci, :w], whh8[:, :, sl],
                                        h8[:, :, o:o + w],
                                        start=False, stop=(not extra_ones),
                                        perf_mode=PM_DR)
                if extra_ones:
                    last = nc.tensor.matmul(Gt[:, ci, :w], ones,
                                            ind_all[:, cols],
                                            start=False, stop=True)

                def sig():
                    _after(nc.scalar.activation(
                        out_sb[:, ci, :w], Gt[:, ci, :w], AF.Sigmoid,
                        bias=bias), last)
                act_pend.append(sig)

            def rmm(ci):
                if ci == 0:
                    bctx["Rt"] = pg.tile([128, 2, NBW], F32, tag="rr", name="Rt")
                    bctx["r_sb"] = sb2.tile([128, 2, NBW], BF16, tag="r_sb", name="r_sb")
                gate_ci(bctx["Rt"], 0, ci, False, bctx["r_sb"],
                        b_r[:, ci:ci + 1])

            def zmm(ci):
                if ci == 0:
                    bctx["Zt"] = pg.tile([128, 2, NBW], F32, tag="zz", name="Zt")
                    bctx["zc_sb"] = sb2.tile([128, 2, NBW], BF16, tag="zc_sb", name="zc_sb")
                gate_ci(bctx["Zt"], 256, ci, frz, bctx["zc_sb"],
                        nb_z[:, ci:ci + 1])

            def nmm():
                Nt = pg.tile([128, 2, NBW], F32, tag="nn")
                lasts = []
                for ci in range(2):
                    sl = slice(512 + ci * 128, 512 + (ci + 1) * 128)
                    lasts.append(nc.tensor.matmul(Nt[:, ci, :w], whh8[:, :, sl],
                                                  h8[:, :, o:o + w],
                                                  start=True, stop=True,
                                                  perf_mode=PM_DR))

                def elem():
                    r_sb, zc_sb = bctx["r_sb"], bctx["zc_sb"]
                    t1 = sb3.tile([128, 2, NBW], BF16, tag="t1_sb")
                    u = sb3.tile([128, 2, NBW], BF16, tag="u_sb")
                    n_sb = sb3.tile([128, 2, NBW], BF16, tag="n_sb")
                    d_sb = sb3.tile([128, 2, NBW], BF16, tag="d_sb")
                    f_sb = sb3.tile([128, 2, NBW], BF16, tag="f_sb")
                    for ci in range(2):
                        _after(nc.vector.scalar_tensor_tensor(
                            t1[:, ci, :w], Nt[:, ci, :w], b_hn[:, ci:ci + 1],
                            r_sb[:, ci, :w], op0=ALU.add, op1=ALU.mult), lasts[ci])
                    nc.vector.tensor_add(u[:, :, :w], t1[:, :, :w],
                                         xn_all[:, :, cols])
                    nc.scalar.activation(n_sb[:, :, :w], u[:, :, :w], AF.Tanh)
                    hsl = h_inter[:, :, o:o + w]
                    nc.vector.tensor_sub(d_sb[:, :, :w], hsl, n_sb[:, :, :w])
                    nc.vector.tensor_mul(f_sb[:, :, :w], zc_sb[:, :, :w], d_sb[:, :, :w])
                    nc.vector.tensor_sub(h8[:, :, o:o + w], hsl, f_sb[:, :, :w])
                    nc.vector.tensor_sub(hsl, hsl, f_sb[:, :, :w])
                    if last_block_of_step.get(t) == bi:
                        kv_ready.extend(kv_after_step.get(t, []))
                dve_pend.append(elem)

            return [lambda: rmm(0), lambda: rmm(1), lambda: zmm(0),
                    lambda: zmm(1), nmm]

        # ---------------- xn pipeline (input proj of n gate, any cols) -------
        def xn_chunk(c):
            a = c * XCW
            wc = min(XCW, TOTAL - a)
            px = pxn.tile([128, 2, XCW], F32, tag="xn")
            m0 = nc.tensor.matmul(px[:, 0, :wc], wihT[:, 512:640], xin[:, a:a + wc],
                                  start=True, stop=False)
            m1 = nc.tensor.matmul(px[:, 1, :wc], wihT[:, 640:768], xin[:, a:a + wc],
                                  start=False, stop=True)
            _coloc([m0, m1])

            def evac():
                for ci in range(2):
                    _after(nc.vector.tensor_scalar_add(
                        xn_all[:, ci, a:a + wc], px[:, ci, :wc],
                        b_in[:, ci:ci + 1]), m1)
            pool_pend.append(evac)

        # ---------------- interleaved attention pieces (borrow xn bank) ------
        k_sb = singles.tile([128, NST, 256], BF16, tag="k_sb")
        v_sb = singles.tile([128, NST, 256], BF16, tag="v_sb")
        qa_sb = singles.tile([128, 2, 128], BF16, tag="qa_sb")
        ka_sb = singles.tile([128, 2, 128], BF16, tag="ka_sb")
        ms_all = singles.tile([S, BPC, 2, S], BF16, tag="ms_all")

        xflat_i = xintra.rearrange("d b s -> d (b s)")
        xp_own = xflat_i[0:127, 0:NTOK]

        def kv_tile_gen(s_):
            # k and v projections share the single xn PSUM bank: evacs must
            # wait for the LAST matmul into the bank (single-port rule).
            # Two pieces (k mms | v mms + both evacs) to keep PE quanta small.
            cols = slice(s_ * 128, (s_ + 1) * 128)
            px = pxn.tile([128, 2, XCW], F32, tag="xn")
            ev = []
            starts = []
            mlast = None
            for bank, (lhs, rhs, bias, osb) in enumerate((
                ([h_inter[:, 0, cols], h_inter[:, 1, cols], rT[0:127, cols]],
                 [W_["ikw0"], W_["ikw1"], W_["ikwx"]], W_["ikb"], k_sb[:, s_, :]),
                ([h_inter[:, 0, cols], h_inter[:, 1, cols], rT[:, cols]],
                 [W_["ivw0"], W_["ivw1"], W_["ivwx"]], W_["ivb"], v_sb[:, s_, :]),
            )):
                p = px[:, bank, :]
                first = True
                for (lt, rt) in zip(lhs, rhs):
                    m = nc.tensor.matmul(p, lt, rt, start=(first and bank == 0),
                                         stop=False)
                    if first:
                        starts.append(m)
                    first = False
                mlast = nc.tensor.matmul(p, ones, bias, start=False,
                                         stop=(bank == 1))
                ev.append((osb, p))
                if bank == 0:
                    yield
            _coloc(starts)
            for osb, p in ev:
                _after(nc.vector.tensor_copy(osb, p), mlast)
            yield

        def qaka():
            px = pxn.tile([128, 2, XCW], F32, tag="xn")
            evs = []
            starts = []
            mlast = None
            for bank, (wn, ob, bias) in enumerate((("aqw", qa_sb, aqb),
                                                   ("akw", ka_sb, akb))):
                ps = px[:, bank, :].rearrange("p (c n) -> p c n", c=2)
                for ci in range(2):
                    mlast = nc.tensor.matmul(ps[:, ci, :],
                                             W_[wn][:, ci * 128:(ci + 1) * 128],
                                             xp_own,
                                             start=(bank == 0 and ci == 0),
                                             stop=(bank == 1 and ci == 1))
                    if ci == 0:
                        starts.append(mlast)
                for ci in range(2):
                    evs.append((ob[:, ci, :], ps[:, ci, :], bias[:, ci:ci + 1]))
            _coloc(starts)
            for ob_, ps_, b_ in evs:
                _after(nc.vector.tensor_scalar_add(ob_, ps_, b_), mlast)

        def intra_chain(bl, hh):
            # scores + causal mask only; exp deferred to the tail (ACT tables)
            px = pxn.tile([128, 2, XCW], F32, tag="xn")
            sca = px[0:S, 0, 0:S]
            m = nc.tensor.matmul(sca, qa_sb[:, hh, bl * S:(bl + 1) * S],
                                 ka_sb[:, hh, bl * S:(bl + 1) * S],
                                 start=True, stop=True)
            _after(nc.vector.tensor_add(ms_all[:, bl, hh, :], sca, W_["cmask"]), m)

        # k/v tiles become final when the active width drops below their cols
        kv_after_step = {}
        for s_ in range(NST):
            ready = max((t for t in range(L) if W[t] > s_ * 128), default=-1)
            kv_after_step.setdefault(ready, []).append(s_)

        blocks = [(t, o, w) for t in range(L) for (o, w) in _blocks_of(W[t])]
        last_block_of_step = {}
        for bi, (t, o, w) in enumerate(blocks):
            last_block_of_step[t] = bi
        mm_pieces = deque()
        for bi, (t, o, w) in enumerate(blocks):
            for p in make_block(bi, t, o, w):
                mm_pieces.append(p)

        # ---------------- emission schedule (round template) ----------------
        # HAM warm-up: dep-free junk matmuls at t=0 (overlaps the DMA wait)
        Iw = pia.tile([128, 32, B], F32, tag="ia")
        jreg = Iw[:, 6:32, :].rearrange("p a b -> p (a b)")
        for _ in range(10):
            nc.tensor.matmul(jreg, jsrc[:, 0:128], jsrc[:, 0:416],
                             start=True, stop=True)

        xprep_chunk(0)

        NMP = len(mm_pieces)
        IST, IEND = 2, 62
        mp_done = 0
        xn_done = 0
        kv_emitted = set()
        sca_rounds = {53: (0, 0), 55: (0, 1), 57: (1, 0), 59: (1, 1)}

        for i in range(S):
            intra_mm_sig(i)
            intra_t1_u(i)
            intra_tail(i)
            if 0 <= i <= 2:
                xprep_chunk(i + 1)
            # xn pipeline: matmuls now, psum evacuation in the DVE slot below
            if xn_done < NCH and xn_done <= i:
                xn_chunk(xn_done)
                xn_done += 1
            # DVE slot: xn evacs + inter elementwise follow-ups
            while pool_pend:
                pool_pend.popleft()()
            while dve_pend:
                dve_pend.popleft()()
            # PE slot: paced inter matmul pieces
            if i >= IST:
                target = min(NMP, (NMP * (i - IST + 1)) // (IEND - IST + 1))
                while mp_done < target:
                    mm_pieces.popleft()()
                    mp_done += 1
            # attention pieces (borrow the xn bank, after the xn pipeline ends)
            if i == 42:
                qaka()
            if i in sca_rounds:
                intra_chain(*sca_rounds[i])
            if i >= 43 and xn_done >= NCH and kv_pieces:
                kv_pieces.popleft()()
            while kv_ready:
                s_ = kv_ready.pop(0)
                if s_ not in kv_emitted:
                    g = kv_tile_gen(s_)
                    kv_pieces.append(lambda g=g: next(g, None))
                    kv_pieces.append(lambda g=g: next(g, None))
                    kv_emitted.add(s_)
            # drain all pending inter sigmas after tanh(i): they execute in
            # the chain's Pool/PE window instead of blocking the ACT visits
            while act_pend:
                act_pend.popleft()()
        while mp_done < NMP:
            mm_pieces.popleft()()
            mp_done += 1
        while dve_pend:
            dve_pend.popleft()()
        while act_pend:
            act_pend.popleft()()
        while kv_pieces:
            kv_pieces.popleft()()
        for s_ in kv_ready + [s_ for s_ in range(NST) if s_ not in kv_emitted]:
            if s_ not in kv_emitted:
                for _ in kv_tile_gen(s_):
                    pass
                kv_emitted.add(s_)

        ia_ps.__exit__(None, None, None)
        xn_ps.__exit__(None, None, None)
        gate_ps.__exit__(None, None, None)

        # ---------------- phase 3: attention + fused final ----------------
        psa = ctx.enter_context(tc.tile_pool(name="psa", bufs=2, space="PSUM"))
        psb = ctx.enter_context(tc.tile_pool(name="psb", bufs=2, space="PSUM"))
        psf = ctx.enter_context(tc.tile_pool(name="psf", bufs=1, space="PSUM"))

        hown = [hT_own[:, ci].rearrange("p b s -> p (b s)") for ci in range(2)]

        def proj(lhs_chunks, rhs_tiles, bias_tile, m_parts=128):
            p = psa.tile([m_parts, 256], F32, tag="proj")
            first = True
            last = None
            for (lt, rt) in zip(lhs_chunks, rhs_tiles):
                last = nc.tensor.matmul(p, lt, rt, start=first, stop=False)
                first = False
            last = nc.tensor.matmul(p, ones[:, 0:m_parts], bias_tile,
                                    start=False, stop=True)
            return p, last

        # PE-heavy projections first (covers the exp table load that follows)
        va_sb = []
        for bl in range(BPC):
            vp, vl = proj([hT_own[:, 0, bl, :], hT_own[:, 1, bl, :], xlast[:, bl, :]],
                          [W_["avw0"], W_["avw1"], W_["avwx"]], W_["avb"], m_parts=S)
            vb = sb2.tile([S, 256], BF16, tag="va_sb")
            _after(nc.vector.tensor_copy(vb, vp), vl)
            va_sb.append(vb)

        q_ps, q_last = proj([hown[0], hown[1], xp_own],
                            [W_["iqw0"], W_["iqw1"], W_["iqwx"]], W_["iqb"])
        q_sb = sb2.tile([128, 256], BF16, tag="q_sb")
        _after(nc.vector.tensor_copy(q_sb, q_ps), q_last)

        qp_sbs = []
        for s_ in range(NST):
            cols = slice(s_ * 128, (s_ + 1) * 128)
            qpp = psa.tile([128, 256], F32, tag="proj")
            m = nc.tensor.matmul(qpp, W_["Pq"][:, cols], q_sb, start=True, stop=True)
            qp_sb = sb3.tile([128, 256], BF16, tag="qp_sb")
            _after(nc.vector.tensor_copy(qp_sb, qpp), m)
            qp_sbs.append(qp_sb)

        # intra-attention softmax chains (exp first -> one table load)
        e_sb = singles.tile([128, NST, 2], BF16, tag="e_sb")
        e32 = singles.tile([128, NST, 2], F32, tag="e32")
        ex_t = []
        for bl in range(BPC):
            for hh in range(2):
                ex = sb3.tile([S, S], BF16, tag=f"ex{bl}{hh}")
                nc.scalar.activation(ex, ms_all[:, bl, hh, :], AF.Exp)
                ex_t.append(ex)
        for s_ in range(NST):
            scratch = sb3.tile([128, 2, 128], BF16, tag="ttr_scratch")
            nc.vector.tensor_mul(scratch,
                                 qp_sbs[s_].rearrange("p (c n) -> p c n", c=2),
                                 k_sb[:, s_, :].rearrange("p (c n) -> p c n", c=2))
            sc = sb3.tile([128, 2, 1], F32, tag="sc")
            nc.vector.tensor_reduce(sc, scratch, axis=AX.X, op=ALU.add)
            nc.scalar.activation(e32[:, s_, :].rearrange("p (c one) -> p c one", c=2),
                                 sc, AF.Exp)
            nc.vector.tensor_copy(e_sb[:, s_, :], e32[:, s_, :])
        paT_all = sb2.tile([S, BPC, 2, S], BF16, tag="paT_all")
        for j, (bl, hh) in enumerate((b_, h_) for b_ in range(BPC) for h_ in range(2)):
            ex = ex_t[j]
            rs = sb3.tile([S, 1], F32, tag="rs")
            nc.vector.tensor_reduce(rs, ex, axis=AX.X, op=ALU.add)
            ri = sb3.tile([S, 1], F32, tag="ri")
            nc.vector.reciprocal(ri, rs)
            pa = sb3.tile([S, S], BF16, tag="pa")
            nc.vector.tensor_scalar_mul(pa, ex, ri)
            ptp = psb.tile([S, S], BF16, tag="tp", name="ptp")
            tm = nc.tensor.transpose(ptp, pa, W_["id128"][0:S, 0:S])
            _after(nc.vector.tensor_copy(paT_all[:, bl, hh, :], ptp), tm)

        # esum per token + weighted values in one PSUM bank
        acc = psf.tile([128, 512], F32, tag="acc")
        esum_ps = acc[:, 256:258]
        o_ps = acc[:, 0:256]
        acc_insts = []
        for s_ in range(NST):
            cols = slice(s_ * 128, (s_ + 1) * 128)
            acc_insts.append(nc.tensor.matmul(
                esum_ps, W_["Pi"][:, cols], e_sb[:, s_, :],
                start=(s_ == 0), stop=False))
        ow_last = None
        for s_ in range(NST):
            cols = slice(s_ * 128, (s_ + 1) * 128)
            vw = sb3.tile([128, 256], BF16, tag="vw")
            for hh in range(2):
                hs = slice(hh * 128, (hh + 1) * 128)
                nc.vector.tensor_scalar_mul(vw[:, hs], v_sb[:, s_, hs],
                                            e32[:, s_, hh:hh + 1])
            ow_last = nc.tensor.matmul(o_ps, W_["Pi"][:, cols], vw,
                                       start=False, stop=(s_ == NST - 1))
            acc_insts.append(ow_last)
        _coloc(acc_insts)
        einv = sb2.tile([128, 2], F32, tag="einv")
        _after(nc.vector.reciprocal(einv, esum_ps), ow_last)
        o_i = sb2.tile([128, 256], BF16, tag="o_i")
        for hh in range(2):
            hs = slice(hh * 128, (hh + 1) * 128)
            _after(nc.vector.tensor_scalar_mul(o_i[:, hs], o_ps[:, hs],
                                               einv[:, hh:hh + 1]), ow_last)

        oiT = sb2.tile([128, 2, 128], BF16, tag="oiT")
        for ci in range(2):
            tp = psb.tile([128, 128], BF16, tag="tp", name="tp")
            tm = nc.tensor.transpose(tp, o_i[:, ci * 128:(ci + 1) * 128], W_["id128"])
            _after(nc.vector.tensor_copy(oiT[:, ci, :], tp), tm)

        oaT = sb2.tile([128, 2, 128], BF16, tag="oaT")
        for bl in range(BPC):
            for hh in range(2):
                op = psb.tile([128, S], F32, tag="tp")
                m = nc.tensor.matmul(op, va_sb[bl][:, hh * 128:(hh + 1) * 128],
                                     paT_all[:, bl, hh, :], start=True, stop=True)
                _after(nc.vector.tensor_copy(oaT[:, hh, bl * S:(bl + 1) * S], op), m)

        # fused final projection
        fo = psf.tile([128, 512], F32, tag="acc", name="fo")[:, 0:256]
        nc.tensor.matmul(fo, oiT[:, 0, :], W_["AiT0"], start=True, stop=False)
        nc.tensor.matmul(fo, oiT[:, 1, :], W_["AiT1"], start=False, stop=False)
        nc.tensor.matmul(fo, oaT[:, 0, :], W_["AaT0"], start=False, stop=False)
        nc.tensor.matmul(fo, oaT[:, 1, :], W_["AaT1"], start=False, stop=False)
        nc.tensor.matmul(fo, hown[0], W_["LhT0"], start=False, stop=False)
        nc.tensor.matmul(fo, hown[1], W_["LhT1"], start=False, stop=False)
        nc.tensor.matmul(fo, xp_own, W_["LxT"], start=False, stop=False)
        fin = nc.tensor.matmul(fo, ones, W_["btot"], start=False, stop=True)
        out_sb = sb2.tile([128, 256], F32, tag="out_sb")
        _after(nc.vector.tensor_copy(out_sb, fo), fin)
        nc.sync.dma_start(out=d_out.ap(), in_=out_sb)


def _build(W, MINACT):
    OFF = [0]
    for t in range(L - 1):
        OFF.append(OFF[t] + W[t])
    TOTAL = OFF[-1] + W[-1]

    nc = bacc.Bacc("TRN2", target_bir_lowering=False, debug=False)
    di = {}

    def inp(name, shape, dt=BF16):
        di[name] = nc.dram_tensor(name, list(shape), dt, kind="ExternalInput")

    inp("xinter", [128, TOTAL])
    inp("xintra", [128, B, S])
    inp("xlast", [1, B, S])
    inp("rT", [128, NSEQ])
    inp("indr", [1, TOTAL])
    inp("wihT", [128, 768])
    inp("whh0T", [128, 768])
    inp("whh1T", [128, 768])
    inp("whh8", [128, 2, 768], F8)
    inp("blobB", [128, BLOBW])
    inp("blobC", [1, 256 * len(_BLOBC_NAMES)])
    inp("blobD", [128, 2 * len(_BLOBD_NAMES)], F32)
    inp("id128e", [128, 128])
    inp("bhnT", [2, 128])
    inp("ind2", [2, 2 * B])

    d_out = nc.dram_tensor("out", [NTOK, 256], F32, kind="ExternalOutput")

    with tile.TileContext(nc) as tc:
        _emit(nc, tc, di, d_out, W, OFF, MINACT)
    nc.compile()
    return nc


# ----------------------------------------------------------------------------
# host-side prep (unchanged from v1 apart from removed tensors)
# ----------------------------------------------------------------------------

def _plan(inter_len):
    lens5 = np.asarray(inter_len, np.int64).reshape(B, S, R)
    orders, lens_sorted = [], []
    act = np.zeros((NCORES, L), np.int64)
    for c in range(NCORES):
        lens = lens5[[2 * c, 2 * c + 1]].reshape(NSEQ)
        order = np.argsort(-lens, kind="stable")
        ls = lens[order]
        orders.append(order)
        lens_sorted.append(ls)
        for t in range(L):
            act[c, t] = int((ls > t).sum())
    W = [min(NSEQ, int(-32 * (-(act[:, t].max()) // 32))) for t in range(L)]
    W = [max(32, w) for w in W]
    for t in range(1, L):
        W[t] = min(W[t], W[t - 1])
    MINACT = [int(act[:, t].min()) for t in range(L)]
    OFF = [0]
    for t in range(L - 1):
        OFF.append(OFF[t] + W[t])
    return orders, lens_sorted, W, MINACT, OFF


def prep_in_maps(inputs):
    inp = {k: np.asarray(v) for k, v in inputs.items()}
    w_ih = f32c(inp["w_ih"])
    w_hh = f32c(inp["w_hh"])
    b_ih = f32c(inp["b_ih"])
    b_hh = f32c(inp["b_hh"])
    b_rz = b_ih[:2 * H] + b_hh[:2 * H]
    sq = np.sqrt(128.0)

    e = np.exp(f32c(inp["wr"])[0, 0] - f32c(inp["wr"])[0, 0].max())
    w01 = e / e.sum()
    ln_w = f32c(inp["ln_w"])
    L_v, L_h, L_x = ln_w[:, :H], ln_w[:, H:2 * H], ln_w[:, 2 * H:]
    Ai = w01[0] * (L_v @ f32c(inp["io_w"]))
    Aa = w01[1] * (L_v @ f32c(inp["ao_w"]))
    btot = f32c(inp["ln_b"]) + L_v @ (w01[0] * f32c(inp["io_b"]) + w01[1] * f32c(inp["ao_b"]))

    iq_w = f32c(inp["iq_w"]) / sq
    iq_b = f32c(inp["iq_b"]) / sq
    aq_w = f32c(inp["aq_w"]) / sq
    aq_b = f32c(inp["aq_b"]) / sq

    def chunks2(m):
        return f32c(np.stack([m[:128], m[128:256]], axis=1))

    orders, lens_sorted, W, MINACT, OFF = _plan(inp["inter_len"])
    TOTAL = OFF[-1] + W[-1]

    x_bs = f32c(inp["intra_x"])
    his5 = f32c(inp["inter_his"]).reshape(B, S, R, L, D)
    r5 = f32c(inp["inter_r"]).reshape(B, S, R, D)

    bw = {
        "iqw0": iq_w.T[0:128], "iqw1": iq_w.T[128:256], "iqwx": iq_w.T[256:383],
        "ikw0": inp["ik_w"].T[0:128], "ikw1": inp["ik_w"].T[128:256],
        "ikwx": inp["ik_w"].T[256:383],
        "ivw0": inp["iv_w"].T[0:128], "ivw1": inp["iv_w"].T[128:256],
        "ivwx": inp["iv_w"].T[256:384],
        "aqw": aq_w.T, "akw": f32c(inp["ak_w"]).T,
        "avw0": inp["av_w"].T[0:128], "avw1": inp["av_w"].T[128:256],
        "AiT0": Ai.T[0:128], "AiT1": Ai.T[128:256],
        "AaT0": Aa.T[0:128], "AaT1": Aa.T[128:256],
        "LhT0": L_h.T[0:128], "LhT1": L_h.T[128:256], "LxT": L_x.T,
        "id128": np.eye(128, dtype=np.float32),
        "cmask": np.where(np.tril(np.ones((S, S), bool)), 0.0, -BIG),
    }
    blobC = np.zeros((1, 256 * len(_BLOBC_NAMES)), np.float32)
    bc = {
        "iqb": iq_b, "ikb": f32c(inp["ik_b"]), "ivb": f32c(inp["iv_b"]),
        "avwx": f32c(inp["av_w"]).T[256], "avb": f32c(inp["av_b"]), "btot": btot,
    }
    for i, nm in enumerate(_BLOBC_NAMES):
        blobC[0, 256 * i:256 * i + len(bc[nm])] = bc[nm]
    blobD = np.zeros((128, 2 * len(_BLOBD_NAMES)), np.float32)
    bd = {
        "b_r": chunks2(b_rz[:H]), "nb_z": chunks2(-b_rz[H:]),
        "b_in": chunks2(b_ih[2 * H:]), "b_hn": chunks2(b_hh[2 * H:]),
        "aqb": chunks2(aq_b), "akb": chunks2(f32c(inp["ak_b"])),
    }
    for i, nm in enumerate(_BLOBD_NAMES):
        blobD[:, 2 * i:2 * i + 2] = bd[nm]

    wihT_h = w_ih.T.copy()
    wihT_h[:, 256:512] *= -1.0
    whhT_h = w_hh.T.copy()
    whhT_h[:, 256:512] *= -1.0
    b_hn_full = b_hh[2 * H:]
    f8c = lambda x: np.ascontiguousarray(
        np.asarray(x, np.float32).astype(ml_dtypes.float8_e4m3))
    shared = dict(
        wihT=bfc(wihT_h),
        whh0T=bfc(whhT_h[0:128]),
        whh1T=bfc(whhT_h[128:256]),
        whh8=f8c(whhT_h.reshape(2, 128, 768).transpose(1, 0, 2)),
        blobC=bfc(blobC),
        blobD=f32c(blobD),
        id128e=bfc(np.eye(128, dtype=np.float32)),
        bhnT=bfc(np.stack([b_hn_full[0:128], b_hn_full[128:256]])),
        ind2=bfc(np.kron(np.eye(2), np.ones((1, B))).reshape(2, 2 * B)),
    )

    in_maps = []
    for c in range(NCORES):
        bsel = [2 * c, 2 * c + 1]
        order = orders[c]
        ls = lens_sorted[c]
        his_cols = his5[bsel].reshape(NSEQ, L, D)[order]
        xint = np.zeros((D, TOTAL), np.float32)
        ind = np.zeros((1, TOTAL), np.float32)
        for t in range(L):
            o, w = OFF[t], W[t]
            xint[:, o:o + w] = his_cols[:w, t, :].T
            ind[0, o:o + w] = -BIG * (t >= ls[:w])
        rTc = r5[bsel].reshape(NSEQ, D)[order].T
        tok_of = order // R
        Pq = np.zeros((128, NSEQ), np.float32)
        Pi = np.zeros((128, NSEQ), np.float32)
        for s_ in range(NST):
            for pl in range(128):
                tok = tok_of[s_ * 128 + pl]
                Pq[tok, s_ * 128 + pl] = 1.0
                Pi[pl, s_ * 128 + tok] = 1.0
        blobB = np.zeros((128, BLOBW), np.float32)
        for nm, (o_, w_) in _BLOB_OFF.items():
            src = {"Pq": Pq, "Pi": Pi}.get(nm)
            if src is None:
                src = bw[nm]
            blobB[0:src.shape[0], o_:o_ + src.shape[1]] = src
        rolled = np.roll(x_bs, -2 * c, axis=0)
        xia = rolled.transpose(2, 0, 1)
        m = dict(shared)
        m.update(
            xinter=bfc(xint),
            xintra=bfc(xia),
            xlast=bfc(xia[127:128]),
            rT=bfc(rTc),
            indr=bfc(ind),
            blobB=bfc(blobB),
        )
        in_maps.append(m)
    return in_maps, W, MINACT


def assemble(core_outs):
    o = np.stack([np.asarray(co, np.float32) for co in core_outs])
    return np.ascontiguousarray(o.reshape(B * S, 256))


_CACHE = {}


def kernel(**inputs) -> np.ndarray:
    in_maps, W, MINACT = prep_in_maps(inputs)
    key = (tuple(W), tuple(MINACT))
    if _CACHE.get("key") != key:
        _CACHE["nc"] = _build(W, MINACT)
        _CACHE["key"] = key
    nc = _CACHE["nc"]
    res = run_bass_kernel_spmd(nc, in_maps, core_ids=list(range(NCORES)))
    return assemble([r["out"] for r in res.results])


# revision 16
# speedup vs baseline: 1.0042x; 1.0042x over previous
"""Trainium2 Bass kernel for nn_CoKT — v2 (chain-optimized rewrite).

Design (per core: 128 own tokens, 768 inter seqs, weights replicated):
- The intra-GRU 64-step serial chain paces the kernel (measured 4.7us/step in
  v1 = 300us).  v2 shortens it: contiguous ping-pong h tiles, z-path split
  (h' = (h - zc*h) + zc*n) so only 2 ops trail tanh, post-sigmoid ops on the
  otherwise-idle GpSimd engine, per-step history stored only for own batches.
- Inter GRU runs in 512-wide blocks with per-ci PSUM banks (6 banks), halving
  ACT/PE instruction counts per column vs 256-wide tiles; h-updates on DVE in
  4x mode (was GpSimd at ~1.1us/op); xn input-projection pipelined through a
  dedicated 1-bank chunk queue with DVE evacuation.
- DMA priority order + split xinter so the chain starts ~3us in (was 34us).
- Phase 3 reordered: PE-heavy projections emitted first to cover the exp
  table load, independent softmax chains interleaved.
"""
import sys
if "/opt/trn_rl_repo" not in sys.path:
    sys.path.insert(0, "/opt/trn_rl_repo")

import numpy as np
import ml_dtypes

import concourse.bacc as bacc
import concourse.mybir as mybir
import concourse.tile as tile
from concourse.tile import add_dep_helper
from concourse.bass_utils import run_bass_kernel_spmd

F32 = mybir.dt.float32
BF16 = mybir.dt.bfloat16
F8 = mybir.dt.float8e4
PM_DR = mybir.MatmulPerfMode.DoubleRow
AF = mybir.ActivationFunctionType
ALU = mybir.AluOpType
AX = mybir.AxisListType

B, S, R, L, D, H = 16, 64, 6, 24, 128, 256
NCORES = 8
BPC = B // NCORES            # 2 batches per core
NTOK = S * BPC               # 128 own tokens
NSEQ = NTOK * R              # 768 inter sequences per core
NBW = 512                    # inter block width (1 PSUM bank per ci)
XCW = 256                    # xn chunk width
BIG = 30000.0
NST = NSEQ // 128            # 6 seq-tiles of 128 in the attention phase

bfc = lambda x: np.ascontiguousarray(np.asarray(x, np.float32).astype(ml_dtypes.bfloat16))
f32c = lambda x: np.ascontiguousarray(np.asarray(x, np.float32))

_BLOB_NAMES = [
    ("iqw0", 256), ("iqw1", 256), ("ikw0", 256), ("ikw1", 256),
    ("ivw0", 256), ("ivw1", 256), ("ivwx", 256), ("avw0", 256), ("avw1", 256),
    ("AiT0", 256), ("AiT1", 256), ("AaT0", 256), ("AaT1", 256),
    ("LhT0", 256), ("LhT1", 256),
    ("iqwx", 256), ("ikwx", 256), ("aqw", 256), ("akw", 256), ("LxT", 256),
    ("id128", 128), ("Pq", NSEQ), ("Pi", NSEQ), ("cmask", S),
]
_BLOB_OFF = {}
_off = 0
for _nm, _w in _BLOB_NAMES:
    _BLOB_OFF[_nm] = (_off, _w)
    _off += _w
BLOBW = _off
_ROWS127 = {"iqwx", "ikwx", "aqw", "akw", "LxT"}

_BLOBC_NAMES = ["iqb", "ikb", "ivb", "avwx", "avb", "btot"]
_BLOBD_NAMES = ["b_r", "nb_z", "b_in", "b_hn", "aqb", "akb"]


def _blocks_of(w, step=NBW):
    out = []
    o = 0
    while o < w:
        out.append((o, min(step, w - o)))
        o += step
    return out


# ----------------------------------------------------------------------------
# device program
# ----------------------------------------------------------------------------

def _coloc(insts):
    first = insts[0]
    for x in insts[1:]:
        add_dep_helper(x.ins, first.ins, sync=True, reason="psum coloc order")


def _after(consumer, last_mm):
    """Reader of a PSUM bank must wait until the PE is done with the bank."""
    add_dep_helper(consumer.ins, last_mm.ins, sync=True, reason="bank read-after-all-mm")


def _emit(nc, tc, di, d_out, W, OFF, MINACT):
    import contextlib
    ctx = contextlib.ExitStack()
    TOTAL = OFF[-1] + W[-1]
    NCH = (TOTAL + XCW - 1) // XCW
    with ctx:
        singles = ctx.enter_context(tc.tile_pool(name="singles", bufs=1))
        sb2 = ctx.enter_context(tc.tile_pool(name="work2", bufs=2))
        sb3 = ctx.enter_context(tc.tile_pool(name="work3", bufs=3))

        def load(name, cols=None):
            d = di[name]
            t = singles.tile(list(d.shape), d.dtype, tag=name)
            if cols is None:
                nc.sync.dma_start(out=t, in_=d.ap())
            else:
                a, b_ = cols
                nc.sync.dma_start(out=t[:, a:b_], in_=d.ap()[:, a:b_])
            return t

        # DMA priority order: intra-scan inputs first so the chain starts early
        wihT = load("wihT")
        xintra = load("xintra")
        blobD = load("blobD")
        bhnT = load("bhnT")
        ind2 = load("ind2")
        whhT = [load("whh0T"), load("whh1T")]
        whh8 = load("whh8")
        id128e = load("id128e")
        XSPLIT = min(1024, TOTAL)
        xin = load("xinter", cols=(0, XSPLIT))
        ind_all = load("indr")
        nc.sync.dma_start(out=xin[:, XSPLIT:TOTAL], in_=di["xinter"].ap()[:, XSPLIT:TOTAL])
        rT = load("rT")
        xlast = load("xlast")
        blobB = singles.tile([128, BLOBW], BF16, tag="blobB")
        nc.sync.dma_start(out=blobB, in_=di["blobB"].ap())
        blobC = singles.tile([1, 256 * len(_BLOBC_NAMES)], BF16, tag="blobC")
        nc.sync.dma_start(out=blobC, in_=di["blobC"].ap())

        bD = {nm: blobD[:, 2 * i:2 * i + 2] for i, nm in enumerate(_BLOBD_NAMES)}
        b_r, nb_z, b_in, b_hn = bD["b_r"], bD["nb_z"], bD["b_in"], bD["b_hn"]
        aqb, akb = bD["aqb"], bD["akb"]
        W_ = {}
        for nm, (o_, w_) in _BLOB_OFF.items():
            rows = 127 if nm in _ROWS127 else (S if nm == "cmask" else 128)
            W_[nm] = blobB[0:rows, o_:o_ + w_]
        for i, nm in enumerate(_BLOBC_NAMES):
            W_[nm] = blobC[:, 256 * i:256 * (i + 1)]
        W_["id128"] = id128e

        ones = singles.tile([1, 128], BF16, tag="ones")
        nc.vector.memset(ones, 1.0)
        jsrc = singles.tile([128, 512], BF16, tag="jsrc")
        nc.vector.memset(jsrc, 0.5)

        xn_all = singles.tile([128, 2, TOTAL], BF16, tag="xn_all")
        xn_intra = singles.tile([128, 2, S, B], BF16, tag="xn_intra")
        xw4 = singles.tile([128, 4, B, S], BF16, tag="xw4")
        hT_own = singles.tile([128, 2, BPC, S], BF16, tag="hT_own")
        hpp = singles.tile([128, 2, 2, B], BF16, tag="hpp")
        nc.vector.memset(hpp, 0.0)
        h_inter = singles.tile([128, 2, NSEQ], BF16, tag="h_inter")
        nc.vector.memset(h_inter, 0.0)
        h8 = singles.tile([128, 2, NSEQ], F8, tag="h8")
        nc.vector.memset(h8, 0.0)

        # PSUM: 6 banks inter gates + 1 bank xn pipeline + 1 bank intra = 8
        gate_ps = tc.tile_pool(name="pg", bufs=1, space="PSUM")
        pg = gate_ps.__enter__()
        xn_ps = tc.tile_pool(name="pxn", bufs=1, space="PSUM")
        pxn = xn_ps.__enter__()
        ia_ps = tc.tile_pool(name="pia", bufs=1, space="PSUM")
        pia = ia_ps.__enter__()

        # ---------------- phase 1: intra input projections, by s-chunk -------
        def xprep_chunk(c):
            s0 = 16 * c
            xv = xintra[:, :, s0:s0 + 16]             # [128, B, 16] (b outer)
            for g in range(4):
                tag = ("rr", "zz")[g // 2]
                gt = pg.tile([128, 2, NBW], F32, tag=tag)
                px = gt[:, g % 2, 0:B * 16]
                m = nc.tensor.matmul(px, wihT[:, g * 128:(g + 1) * 128],
                                     xv, start=True, stop=True)
                bias = b_r[:, g:g + 1] if g < 2 else nb_z[:, g - 2:g - 1]
                ev = nc.vector.tensor_scalar_add(
                    xw4[:, g, :, s0:s0 + 16],
                    px.rearrange("p (b s) -> p b s", b=B), bias)
                _after(ev, m)
            nt = pg.tile([128, 2, NBW], F32, tag="nn")
            xv_sb = xintra[:, :, s0:s0 + 16].rearrange("p b s -> p s b")
            for ci in range(2):
                px = nt[:, ci, 0:B * 16]
                m = nc.tensor.matmul(px, wihT[:, 512 + ci * 128:640 + ci * 128],
                                     xv_sb, start=True, stop=True)
                ev = nc.vector.tensor_scalar_add(
                    xn_intra[:, ci, s0:s0 + 16, :],
                    px.rearrange("p (s b) -> p s b", s=16), b_in[:, ci:ci + 1])
                _after(ev, m)


        # pending-work queues for the round template
        from collections import deque
        act_pend = deque()    # inter sigmoid pieces (fill the chain's ACT gaps)
        dve_pend = deque()    # inter stt/u/tanh/d/f/h combined pieces
        pool_pend = deque()   # xn evacuations (run in Pool's idle tail)
        kv_ready = []
        kv_pieces = deque()

        # ---------------- intra GRU step (the critical chain) ----------------
        ictx = {}

        def intra_mm_sig(s):
            hprev = hpp[:, (s + 1) % 2]
            I = pia.tile([128, 32, B], F32, tag="ia")
            ictx["I"] = I
            ia = I[:, 0:4, :]
            ib = I[:, 4:6, :]
            id_mm = nc.tensor.matmul(ia.rearrange("p g b -> p (g b)"), id128e,
                                     xw4[:, :, :, s].rearrange("p g b -> p (g b)"),
                                     start=True, stop=False)
            insts = [id_mm]
            for g in range(4):
                sl = slice(g * 128, (g + 1) * 128)
                insts.append(nc.tensor.matmul(ia[:, g, :], whhT[0][:, sl],
                                              hprev[:, 0, :], start=False, stop=False))
                nc.tensor.matmul(ia[:, g, :], whhT[1][:, sl], hprev[:, 1, :],
                                 start=False, stop=False)
            for ci in range(2):
                sl = slice(512 + ci * 128, 512 + (ci + 1) * 128)
                nc.tensor.matmul(ib[:, ci, :], whhT[0][:, sl], hprev[:, 0, :],
                                 start=False, stop=False)
                nc.tensor.matmul(ib[:, ci, :], whhT[1][:, sl], hprev[:, 1, :],
                                 start=False, stop=False)
            fold = nc.tensor.matmul(ib.rearrange("p c b -> p (c b)"), bhnT,
                                    ind2, start=False, stop=True)
            _coloc(insts)
            rz4 = sb2.tile([128, 4, B], BF16, tag="irz4")
            sig = nc.scalar.activation(rz4, ia, AF.Sigmoid)
            _after(sig, fold)
            ictx.update(hprev=hprev, ib=ib, fold=fold, rz4=rz4)

        def intra_t1_u(s):
            t1 = sb2.tile([128, 2, B], BF16, tag="it1")
            u = sb2.tile([128, 2, B], BF16, tag="iu")
            _after(nc.vector.tensor_mul(t1, ictx["ib"], ictx["rz4"][:, 0:2, :]),
                   ictx["fold"])
            nc.vector.tensor_add(u, t1, xn_intra[:, :, s, :])
            ictx["u"] = u

        def intra_tail(s):
            hprev, rz4 = ictx["hprev"], ictx["rz4"]
            hnew = hpp[:, s % 2]
            zc = rz4[:, 2:4, :]
            n_sb = sb2.tile([128, 2, B], BF16, tag="in")
            p1a = sb2.tile([128, 2, B], BF16, tag="ip1a")
            p1 = sb2.tile([128, 2, B], BF16, tag="ip1")
            q = sb2.tile([128, 2, B], BF16, tag="iq")
            ictx["tanh"] = nc.scalar.activation(n_sb, ictx["u"], AF.Tanh)
            # off-chain z-path on Pool (runs while ACT does tanh)
            nc.gpsimd.tensor_mul(p1a, zc, hprev)
            nc.gpsimd.tensor_sub(p1, hprev, p1a)
            # chain tail on Pool
            nc.gpsimd.tensor_mul(q, zc, n_sb)
            nc.gpsimd.tensor_add(hnew, p1, q)
            # own-batch history (off-chain)
            nc.gpsimd.tensor_copy(hT_own[:, :, :, s], hnew[:, :, 0:BPC])

        # ---------------- inter GRU block: mm pieces + deferred follow-ups ---
        def make_block(bi, t, o, w):
            frz = (o + w) > MINACT[t]
            cols = slice(OFF[t] + o, OFF[t] + o + w)
            bctx = {}

            def gate_ci(Gt, goff, ci, extra_ones, out_sb, bias):
                sl = slice(goff + ci * 128, goff + (ci + 1) * 128)
                nc.tensor.matmul(Gt[:, ci, :w], wihT[:, sl], xin[:, cols],
                                 start=True, stop=False)
                last = nc.tensor.matmul(Gt[:, ci, :w], whh8[:, :, sl],
                                        h8[:, :, o:o + w],
                                        start=False, stop=(not extra_ones),
                                        perf_mode=PM_DR)
                if extra_ones:
                    last = nc.tensor.matmul(Gt[:, ci, :w], ones,
                                            ind_all[:, cols],
                                            start=False, stop=True)

                def sig():
                    _after(nc.scalar.activation(
                        out_sb[:, ci, :w], Gt[:, ci, :w], AF.Sigmoid,
                        bias=bias), last)
                act_pend.append(sig)

            def rmm(ci):
                if ci == 0:
                    bctx["Rt"] = pg.tile([128, 2, NBW], F32, tag="rr", name="Rt")
                    bctx["r_sb"] = sb2.tile([128, 2, NBW], BF16, tag="r_sb", name="r_sb")
                gate_ci(bctx["Rt"], 0, ci, False, bctx["r_sb"],
                        b_r[:, ci:ci + 1])

            def zmm(ci):
                if ci == 0:
                    bctx["Zt"] = pg.tile([128, 2, NBW], F32, tag="zz", name="Zt")
                    bctx["zc_sb"] = sb2.tile([128, 2, NBW], BF16, tag="zc_sb", name="zc_sb")
                gate_ci(bctx["Zt"], 256, ci, frz, bctx["zc_sb"],
                        nb_z[:, ci:ci + 1])

            def nmm():
                Nt = pg.tile([128, 2, NBW], F32, tag="nn")
                lasts = []
                for ci in range(2):
                    sl = slice(512 + ci * 128, 512 + (ci + 1) * 128)
                    lasts.append(nc.tensor.matmul(Nt[:, ci, :w], whh8[:, :, sl],
                                                  h8[:, :, o:o + w],
                                                  start=True, stop=True,
                                                  perf_mode=PM_DR))

                def elem():
                    r_sb, zc_sb = bctx["r_sb"], bctx["zc_sb"]
                    t1 = sb3.tile([128, 2, NBW], BF16, tag="t1_sb")
                    u = sb3.tile([128, 2, NBW], BF16, tag="u_sb")
                    n_sb = sb3.tile([128, 2, NBW], BF16, tag="n_sb")
                    d_sb = sb3.tile([128, 2, NBW], BF16, tag="d_sb")
                    f_sb = sb3.tile([128, 2, NBW], BF16, tag="f_sb")
                    for ci in range(2):
                        _after(nc.vector.scalar_tensor_tensor(
                            t1[:, ci, :w], Nt[:, ci, :w], b_hn[:, ci:ci + 1],
                            r_sb[:, ci, :w], op0=ALU.add, op1=ALU.mult), lasts[ci])
                    nc.vector.tensor_add(u[:, :, :w], t1[:, :, :w],
                                         xn_all[:, :, cols])
                    nc.scalar.activation(n_sb[:, :, :w], u[:, :, :w], AF.Tanh)
                    hsl = h_inter[:, :, o:o + w]
                    nc.vector.tensor_sub(d_sb[:, :, :w], hsl, n_sb[:, :, :w])
                    nc.vector.tensor_mul(f_sb[:, :, :w], zc_sb[:, :, :w], d_sb[:, :, :w])
                    nc.vector.tensor_sub(h8[:, :, o:o + w], hsl, f_sb[:, :, :w])
                    nc.vector.tensor_sub(hsl, hsl, f_sb[:, :, :w])
                    if last_block_of_step.get(t) == bi:
                        kv_ready.extend(kv_after_step.get(t, []))
                dve_pend.append(elem)

            return [lambda: rmm(0), lambda: rmm(1), lambda: zmm(0),
                    lambda: zmm(1), nmm]

        # ---------------- xn pipeline (input proj of n gate, any cols) -------
        def xn_chunk(c):
            a = c * XCW
            wc = min(XCW, TOTAL - a)
            px = pxn.tile([128, 2, XCW], F32, tag="xn")
            m0 = nc.tensor.matmul(px[:, 0, :wc], wihT[:, 512:640], xin[:, a:a + wc],
                                  start=True, stop=False)
            m1 = nc.tensor.matmul(px[:, 1, :wc], wihT[:, 640:768], xin[:, a:a + wc],
                                  start=False, stop=True)
            _coloc([m0, m1])

            def evac():
                for ci in range(2):
                    _after(nc.vector.tensor_scalar_add(
                        xn_all[:, ci, a:a + wc], px[:, ci, :wc],
                        b_in[:, ci:ci + 1]), m1)
            pool_pend.append(evac)

        # ---------------- interleaved attention pieces (borrow xn bank) ------
        k_sb = singles.tile([128, NST, 256], BF16, tag="k_sb")
        v_sb = singles.tile([128, NST, 256], BF16, tag="v_sb")
        qa_sb = singles.tile([128, 2, 128], BF16, tag="qa_sb")
        ka_sb = singles.tile([128, 2, 128], BF16, tag="ka_sb")
        ms_all = singles.tile([S, BPC, 2, S], BF16, tag="ms_all")

        xflat_i = xintra.rearrange("d b s -> d (b s)")
        xp_own = xflat_i[0:127, 0:NTOK]

        def kv_tile_gen(s_):
            # k and v projections share the single xn PSUM bank: evacs must
            # wait for the LAST matmul into the bank (single-port rule).
            # Two pieces (k mms | v mms + both evacs) to keep PE quanta small.
            cols = slice(s_ * 128, (s_ + 1) * 128)
            px = pxn.tile([128, 2, XCW], F32, tag="xn")
            ev = []
            starts = []
            mlast = None
            for bank, (lhs, rhs, bias, osb) in enumerate((
                ([h_inter[:, 0, cols], h_inter[:, 1, cols], rT[0:127, cols]],
                 [W_["ikw0"], W_["ikw1"], W_["ikwx"]], W_["ikb"], k_sb[:, s_, :]),
                ([h_inter[:, 0, cols], h_inter[:, 1, cols], rT[:, cols]],
                 [W_["ivw0"], W_["ivw1"], W_["ivwx"]], W_["ivb"], v_sb[:, s_, :]),
            )):
                p = px[:, bank, :]
                first = True
                for (lt, rt) in zip(lhs, rhs):
                    m = nc.tensor.matmul(p, lt, rt, start=(first and bank == 0),
                                         stop=False)
                    if first:
                        starts.append(m)
                    first = False
                mlast = nc.tensor.matmul(p, ones, bias, start=False,
                                         stop=(bank == 1))
                ev.append((osb, p))
                if bank == 0:
                    yield
            _coloc(starts)
            for osb, p in ev:
                _after(nc.vector.tensor_copy(osb, p), mlast)
            yield

        def qaka():
            px = pxn.tile([128, 2, XCW], F32, tag="xn")
            evs = []
            starts = []
            mlast = None
            for bank, (wn, ob, bias) in enumerate((("aqw", qa_sb, aqb),
                                                   ("akw", ka_sb, akb))):
                ps = px[:, bank, :].rearrange("p (c n) -> p c n", c=2)
                for ci in range(2):
                    mlast = nc.tensor.matmul(ps[:, ci, :],
                                             W_[wn][:, ci * 128:(ci + 1) * 128],
                                             xp_own,
                                             start=(bank == 0 and ci == 0),
                                             stop=(bank == 1 and ci == 1))
                    if ci == 0:
                        starts.append(mlast)
                for ci in range(2):
                    evs.append((ob[:, ci, :], ps[:, ci, :], bias[:, ci:ci + 1]))
            _coloc(starts)
            for ob_, ps_, b_ in evs:
                _after(nc.vector.tensor_scalar_add(ob_, ps_, b_), mlast)

        def intra_chain(bl, hh):
            # scores + causal mask only; exp deferred to the tail (ACT tables)
            px = pxn.tile([128, 2, XCW], F32, tag="xn")
            sca = px[0:S, 0, 0:S]
            m = nc.tensor.matmul(sca, qa_sb[:, hh, bl * S:(bl + 1) * S],
                                 ka_sb[:, hh, bl * S:(bl + 1) * S],
                                 start=True, stop=True)
            _after(nc.vector.tensor_add(ms_all[:, bl, hh, :], sca, W_["cmask"]), m)

        # k/v tiles become final when the active width drops below their cols
        kv_after_step = {}
        for s_ in range(NST):
            ready = max((t for t in range(L) if W[t] > s_ * 128), default=-1)
            kv_after_step.setdefault(ready, []).append(s_)

        blocks = [(t, o, w) for t in range(L) for (o, w) in _blocks_of(W[t])]
        last_block_of_step = {}
        for bi, (t, o, w) in enumerate(blocks):
            last_block_of_step[t] = bi
        mm_pieces = deque()
        for bi, (t, o, w) in enumerate(blocks):
            for p in make_block(bi, t, o, w):
                mm_pieces.append(p)

        # ---------------- emission schedule (round template) ----------------
        # HAM warm-up: dep-free junk matmuls at t=0 (overlaps the DMA wait)
        Iw = pia.tile([128, 32, B], F32, tag="ia")
        jreg = Iw[:, 6:32, :].rearrange("p a b -> p (a b)")
        for _ in range(10):
            nc.tensor.matmul(jreg, jsrc[:, 0:128], jsrc[:, 0:416],
                             start=True, stop=True)

        xprep_chunk(0)

        NMP = len(mm_pieces)
        IST, IEND = 2, 59
        mp_done = 0
        xn_done = 0
        kv_emitted = set()
        sca_rounds = {45: (0, 0), 47: (0, 1), 49: (1, 0), 51: (1, 1)}

        for i in range(S):
            intra_mm_sig(i)
            intra_t1_u(i)
            intra_tail(i)
            if 0 <= i <= 2:
                xprep_chunk(i + 1)
            # xn pipeline: matmuls now, psum evacuation in the DVE slot below
            if xn_done < NCH and xn_done <= i:
                xn_chunk(xn_done)
                xn_done += 1
            # DVE slot: xn evacs + inter elementwise follow-ups
            while pool_pend:
                pool_pend.popleft()()
            while dve_pend:
                dve_pend.popleft()()
            # PE slot: paced inter matmul pieces
            if i >= IST:
                target = min(NMP, (NMP * (i - IST + 1)) // (IEND - IST + 1))
                while mp_done < target:
                    mm_pieces.popleft()()
                    mp_done += 1
            # attention pieces (borrow the xn bank, after the xn pipeline ends)
            if i == 42:
                qaka()
            if i in sca_rounds:
                intra_chain(*sca_rounds[i])
            if i >= 43 and xn_done >= NCH and kv_pieces:
                kv_pieces.popleft()()
            while kv_ready:
                s_ = kv_ready.pop(0)
                if s_ not in kv_emitted:
                    g = kv_tile_gen(s_)
                    kv_pieces.append(lambda g=g: next(g, None))
                    kv_pieces.append(lambda g=g: next(g, None))
                    kv_emitted.add(s_)
            # drain all pending inter sigmas after tanh(i): they execute in
            # the chain's Pool/PE window instead of blocking the ACT visits
            while act_pend:
                act_pend.popleft()()
        while mp_done < NMP:
            mm_pieces.popleft()()
            mp_done += 1
        while dve_pend:
            dve_pend.popleft()()
        while act_pend:
            act_pend.popleft()()
        while kv_pieces:
            kv_pieces.popleft()()
        for s_ in kv_ready + [s_ for s_ in range(NST) if s_ not in kv_emitted]:
            if s_ not in kv_emitted:
                for _ in kv_tile_gen(s_):
                    pass
                kv_emitted.add(s_)

        ia_ps.__exit__(None, None, None)
        xn_ps.__exit__(None, None, None)
        gate_ps.__exit__(None, None, None)

        # ---------------- phase 3: attention + fused final ----------------
        psa = ctx.enter_context(tc.tile_pool(name="psa", bufs=2, space="PSUM"))
        psb = ctx.enter_context(tc.tile_pool(name="psb", bufs=2, space="PSUM"))
        psf = ctx.enter_context(tc.tile_pool(name="psf", bufs=1, space="PSUM"))

        hown = [hT_own[:, ci].rearrange("p b s -> p (b s)") for ci in range(2)]

        def proj(lhs_chunks, rhs_tiles, bias_tile, m_parts=128):
            p = psa.tile([m_parts, 256], F32, tag="proj")
            first = True
            last = None
            for (lt, rt) in zip(lhs_chunks, rhs_tiles):
                last = nc.tensor.matmul(p, lt, rt, start=first, stop=False)
                first = False
            last = nc.tensor.matmul(p, ones[:, 0:m_parts], bias_tile,
                                    start=False, stop=True)
            return p, last

        # PE-heavy projections first (covers the exp table load that follows)
        va_sb = []
        for bl in range(BPC):
            vp, vl = proj([hT_own[:, 0, bl, :], hT_own[:, 1, bl, :], xlast[:, bl, :]],
                          [W_["avw0"], W_["avw1"], W_["avwx"]], W_["avb"], m_parts=S)
            vb = sb2.tile([S, 256], BF16, tag="va_sb")
            _after(nc.vector.tensor_copy(vb, vp), vl)
            va_sb.append(vb)

        q_ps, q_last = proj([hown[0], hown[1], xp_own],
                            [W_["iqw0"], W_["iqw1"], W_["iqwx"]], W_["iqb"])
        q_sb = sb2.tile([128, 256], BF16, tag="q_sb")
        _after(nc.vector.tensor_copy(q_sb, q_ps), q_last)

        qp_sbs = []
        for s_ in range(NST):
            cols = slice(s_ * 128, (s_ + 1) * 128)
            qpp = psa.tile([128, 256], F32, tag="proj")
            m = nc.tensor.matmul(qpp, W_["Pq"][:, cols], q_sb, start=True, stop=True)
            qp_sb = sb3.tile([128, 256], BF16, tag="qp_sb")
            _after(nc.vector.tensor_copy(qp_sb, qpp), m)
            qp_sbs.append(qp_sb)

        # intra-attention softmax chains (exp first -> one table load)
        e_sb = singles.tile([128, NST, 2], BF16, tag="e_sb")
        e32 = singles.tile([128, NST, 2], F32, tag="e32")
        ex_t = []
        for bl in range(BPC):
            for hh in range(2):
                ex = sb3.tile([S, S], BF16, tag=f"ex{bl}{hh}")
                nc.scalar.activation(ex, ms_all[:, bl, hh, :], AF.Exp)
                ex_t.append(ex)
        for s_ in range(NST):
            scratch = sb3.tile([128, 2, 128], BF16, tag="ttr_scratch")
            nc.vector.tensor_mul(scratch,
                                 qp_sbs[s_].rearrange("p (c n) -> p c n", c=2),
                                 k_sb[:, s_, :].rearrange("p (c n) -> p c n", c=2))
            sc = sb3.tile([128, 2, 1], F32, tag="sc")
            nc.vector.tensor_reduce(sc, scratch, axis=AX.X, op=ALU.add)
            nc.scalar.activation(e32[:, s_, :].rearrange("p (c one) -> p c one", c=2),
                                 sc, AF.Exp)
            nc.vector.tensor_copy(e_sb[:, s_, :], e32[:, s_, :])
        paT_all = sb2.tile([S, BPC, 2, S], BF16, tag="paT_all")
        for j, (bl, hh) in enumerate((b_, h_) for b_ in range(BPC) for h_ in range(2)):
            ex = ex_t[j]
            rs = sb3.tile([S, 1], F32, tag="rs")
            nc.vector.tensor_reduce(rs, ex, axis=AX.X, op=ALU.add)
            ri = sb3.tile([S, 1], F32, tag="ri")
            nc.vector.reciprocal(ri, rs)
            pa = sb3.tile([S, S], BF16, tag="pa")
            nc.vector.tensor_scalar_mul(pa, ex, ri)
            ptp = psb.tile([S, S], BF16, tag="tp", name="ptp")
            tm = nc.tensor.transpose(ptp, pa, W_["id128"][0:S, 0:S])
            _after(nc.vector.tensor_copy(paT_all[:, bl, hh, :], ptp), tm)

        # esum per token + weighted values in one PSUM bank
        acc = psf.tile([128, 512], F32, tag="acc")
        esum_ps = acc[:, 256:258]
        o_ps = acc[:, 0:256]
        acc_insts = []
        for s_ in range(NST):
            cols = slice(s_ * 128, (s_ + 1) * 128)
            acc_insts.append(nc.tensor.matmul(
                esum_ps, W_["Pi"][:, cols], e_sb[:, s_, :],
                start=(s_ == 0), stop=False))
        ow_last = None
        for s_ in range(NST):
            cols = slice(s_ * 128, (s_ + 1) * 128)
            vw = sb3.tile([128, 256], BF16, tag="vw")
            for hh in range(2):
                hs = slice(hh * 128, (hh + 1) * 128)
                nc.vector.tensor_scalar_mul(vw[:, hs], v_sb[:, s_, hs],
                                            e32[:, s_, hh:hh + 1])
            ow_last = nc.tensor.matmul(o_ps, W_["Pi"][:, cols], vw,
                                       start=False, stop=(s_ == NST - 1))
            acc_insts.append(ow_last)
        _coloc(acc_insts)
        einv = sb2.tile([128, 2], F32, tag="einv")
        _after(nc.vector.reciprocal(einv, esum_ps), ow_last)
        o_i = sb2.tile([128, 256], BF16, tag="o_i")
        for hh in range(2):
            hs = slice(hh * 128, (hh + 1) * 128)
            _after(nc.vector.tensor_scalar_mul(o_i[:, hs], o_ps[:, hs],
                                               einv[:, hh:hh + 1]), ow_last)

        oiT = sb2.tile([128, 2, 128], BF16, tag="oiT")
        for ci in range(2):
            tp = psb.tile([128, 128], BF16, tag="tp", name="tp")
            tm = nc.tensor.transpose(tp, o_i[:, ci * 128:(ci + 1) * 128], W_["id128"])
            _after(nc.vector.tensor_copy(oiT[:, ci, :], tp), tm)

        oaT = sb2.tile([128, 2, 128], BF16, tag="oaT")
        for bl in range(BPC):
            for hh in range(2):
                op = psb.tile([128, S], F32, tag="tp")
                m = nc.tensor.matmul(op, va_sb[bl][:, hh * 128:(hh + 1) * 128],
                                     paT_all[:, bl, hh, :], start=True, stop=True)
                _after(nc.vector.tensor_copy(oaT[:, hh, bl * S:(bl + 1) * S], op), m)

        # fused final projection
        fo = psf.tile([128, 512], F32, tag="acc", name="fo")[:, 0:256]
        nc.tensor.matmul(fo, oiT[:, 0, :], W_["AiT0"], start=True, stop=False)
        nc.tensor.matmul(fo, oiT[:, 1, :], W_["AiT1"], start=False, stop=False)
        nc.tensor.matmul(fo, oaT[:, 0, :], W_["AaT0"], start=False, stop=False)
        nc.tensor.matmul(fo, oaT[:, 1, :], W_["AaT1"], start=False, stop=False)
        nc.tensor.matmul(fo, hown[0], W_["LhT0"], start=False, stop=False)
        nc.tensor.matmul(fo, hown[1], W_["LhT1"], start=False, stop=False)
        nc.tensor.matmul(fo, xp_own, W_["LxT"], start=False, stop=False)
        fin = nc.tensor.matmul(fo, ones, W_["btot"], start=False, stop=True)
        out_sb = sb2.tile([128, 256], F32, tag="out_sb")
        _after(nc.vector.tensor_copy(out_sb, fo), fin)
        nc.sync.dma_start(out=d_out.ap(), in_=out_sb)


def _build(W, MINACT):
    OFF = [0]
    for t in range(L - 1):
        OFF.append(OFF[t] + W[t])
    TOTAL = OFF[-1] + W[-1]

    nc = bacc.Bacc("TRN2", target_bir_lowering=False, debug=False)
    di = {}

    def inp(name, shape, dt=BF16):
        di[name] = nc.dram_tensor(name, list(shape), dt, kind="ExternalInput")

    inp("xinter", [128, TOTAL])
    inp("xintra", [128, B, S])
    inp("xlast", [1, B, S])
    inp("rT", [128, NSEQ])
    inp("indr", [1, TOTAL])
    inp("wihT", [128, 768])
    inp("whh0T", [128, 768])
    inp("whh1T", [128, 768])
    inp("whh8", [128, 2, 768], F8)
    inp("blobB", [128, BLOBW])
    inp("blobC", [1, 256 * len(_BLOBC_NAMES)])
    inp("blobD", [128, 2 * len(_BLOBD_NAMES)], F32)
    inp("id128e", [128, 128])
    inp("bhnT", [2, 128])
    inp("ind2", [2, 2 * B])

    d_out = nc.dram_tensor("out", [NTOK, 256], F32, kind="ExternalOutput")

    with tile.TileContext(nc) as tc:
        _emit(nc, tc, di, d_out, W, OFF, MINACT)
    nc.compile()
    return nc


# ----------------------------------------------------------------------------
# host-side prep (unchanged from v1 apart from removed tensors)
# ----------------------------------------------------------------------------

def _plan(inter_len):
    lens5 = np.asarray(inter_len, np.int64).reshape(B, S, R)
    orders, lens_sorted = [], []
    act = np.zeros((NCORES, L), np.int64)
    for c in range(NCORES):
        lens = lens5[[2 * c, 2 * c + 1]].reshape(NSEQ)
        order = np.argsort(-lens, kind="stable")
        ls = lens[order]
        orders.append(order)
        lens_sorted.append(ls)
        for t in range(L):
            act[c, t] = int((ls > t).sum())
    W = [min(NSEQ, int(-32 * (-(act[:, t].max()) // 32))) for t in range(L)]
    W = [max(32, w) for w in W]
    for t in range(1, L):
        W[t] = min(W[t], W[t - 1])
    MINACT = [int(act[:, t].min()) for t in range(L)]
    OFF = [0]
    for t in range(L - 1):
        OFF.append(OFF[t] + W[t])
    return orders, lens_sorted, W, MINACT, OFF


def prep_in_maps(inputs):
    inp = {k: np.asarray(v) for k, v in inputs.items()}
    w_ih = f32c(inp["w_ih"])
    w_hh = f32c(inp["w_hh"])
    b_ih = f32c(inp["b_ih"])
    b_hh = f32c(inp["b_hh"])
    b_rz = b_ih[:2 * H] + b_hh[:2 * H]
    sq = np.sqrt(128.0)

    e = np.exp(f32c(inp["wr"])[0, 0] - f32c(inp["wr"])[0, 0].max())
    w01 = e / e.sum()
    ln_w = f32c(inp["ln_w"])
    L_v, L_h, L_x = ln_w[:, :H], ln_w[:, H:2 * H], ln_w[:, 2 * H:]
    Ai = w01[0] * (L_v @ f32c(inp["io_w"]))
    Aa = w01[1] * (L_v @ f32c(inp["ao_w"]))
    btot = f32c(inp["ln_b"]) + L_v @ (w01[0] * f32c(inp["io_b"]) + w01[1] * f32c(inp["ao_b"]))

    iq_w = f32c(inp["iq_w"]) / sq
    iq_b = f32c(inp["iq_b"]) / sq
    aq_w = f32c(inp["aq_w"]) / sq
    aq_b = f32c(inp["aq_b"]) / sq

    def chunks2(m):
        return f32c(np.stack([m[:128], m[128:256]], axis=1))

    orders, lens_sorted, W, MINACT, OFF = _plan(inp["inter_len"])
    TOTAL = OFF[-1] + W[-1]

    x_bs = f32c(inp["intra_x"])
    his5 = f32c(inp["inter_his"]).reshape(B, S, R, L, D)
    r5 = f32c(inp["inter_r"]).reshape(B, S, R, D)

    bw = {
        "iqw0": iq_w.T[0:128], "iqw1": iq_w.T[128:256], "iqwx": iq_w.T[256:383],
        "ikw0": inp["ik_w"].T[0:128], "ikw1": inp["ik_w"].T[128:256],
        "ikwx": inp["ik_w"].T[256:383],
        "ivw0": inp["iv_w"].T[0:128], "ivw1": inp["iv_w"].T[128:256],
        "ivwx": inp["iv_w"].T[256:384],
        "aqw": aq_w.T, "akw": f32c(inp["ak_w"]).T,
        "avw0": inp["av_w"].T[0:128], "avw1": inp["av_w"].T[128:256],
        "AiT0": Ai.T[0:128], "AiT1": Ai.T[128:256],
        "AaT0": Aa.T[0:128], "AaT1": Aa.T[128:256],
        "LhT0": L_h.T[0:128], "LhT1": L_h.T[128:256], "LxT": L_x.T,
        "id128": np.eye(128, dtype=np.float32),
        "cmask": np.where(np.tril(np.ones((S, S), bool)), 0.0, -BIG),
    }
    blobC = np.zeros((1, 256 * len(_BLOBC_NAMES)), np.float32)
    bc = {
        "iqb": iq_b, "ikb": f32c(inp["ik_b"]), "ivb": f32c(inp["iv_b"]),
        "avwx": f32c(inp["av_w"]).T[256], "avb": f32c(inp["av_b"]), "btot": btot,
    }
    for i, nm in enumerate(_BLOBC_NAMES):
        blobC[0, 256 * i:256 * i + len(bc[nm])] = bc[nm]
    blobD = np.zeros((128, 2 * len(_BLOBD_NAMES)), np.float32)
    bd = {
        "b_r": chunks2(b_rz[:H]), "nb_z": chunks2(-b_rz[H:]),
        "b_in": chunks2(b_ih[2 * H:]), "b_hn": chunks2(b_hh[2 * H:]),
        "aqb": chunks2(aq_b), "akb": chunks2(f32c(inp["ak_b"])),
    }
    for i, nm in enumerate(_BLOBD_NAMES):
        blobD[:, 2 * i:2 * i + 2] = bd[nm]

    wihT_h = w_ih.T.copy()
    wihT_h[:, 256:512] *= -1.0
    whhT_h = w_hh.T.copy()
    whhT_h[:, 256:512] *= -1.0
    b_hn_full = b_hh[2 * H:]
    f8c = lambda x: np.ascontiguousarray(
        np.asarray(x, np.float32).astype(ml_dtypes.float8_e4m3))
    shared = dict(
        wihT=bfc(wihT_h),
        whh0T=bfc(whhT_h[0:128]),
        whh1T=bfc(whhT_h[128:256]),
        whh8=f8c(whhT_h.reshape(2, 128, 768).transpose(1, 0, 2)),
        blobC=bfc(blobC),
        blobD=f32c(blobD),
        id128e=bfc(np.eye(128, dtype=np.float32)),
        bhnT=bfc(np.stack([b_hn_full[0:128], b_hn_full[128:256]])),
        ind2=bfc(np.kron(np.eye(2), np.ones((1, B))).reshape(2, 2 * B)),
    )

    in_maps = []
    for c in range(NCORES):
        bsel = [2 * c, 2 * c + 1]
        order = orders[c]
        ls = lens_sorted[c]
        his_cols = his5[bsel].reshape(NSEQ, L, D)[order]
        xint = np.zeros((D, TOTAL), np.float32)
        ind = np.zeros((1, TOTAL), np.float32)
        for t in range(L):
            o, w = OFF[t], W[t]
            xint[:, o:o + w] = his_cols[:w, t, :].T
            ind[0, o:o + w] = -BIG * (t >= ls[:w])
        rTc = r5[bsel].reshape(NSEQ, D)[order].T
        tok_of = order // R
        Pq = np.zeros((128, NSEQ), np.float32)
        Pi = np.zeros((128, NSEQ), np.float32)
        for s_ in range(NST):
            for pl in range(128):
                tok = tok_of[s_ * 128 + pl]
                Pq[tok, s_ * 128 + pl] = 1.0
                Pi[pl, s_ * 128 + tok] = 1.0
        blobB = np.zeros((128, BLOBW), np.float32)
        for nm, (o_, w_) in _BLOB_OFF.items():
            src = {"Pq": Pq, "Pi": Pi}.get(nm)
            if src is None:
                src = bw[nm]
            blobB[0:src.shape[0], o_:o_ + src.shape[1]] = src
        rolled = np.roll(x_bs, -2 * c, axis=0)
        xia = rolled.transpose(2, 0, 1)
        m = dict(shared)
        m.update(
            xinter=bfc(xint),
            xintra=bfc(xia),
            xlast=bfc(xia[127:128]),
            rT=bfc(rTc),
            indr=bfc(ind),
            blobB=bfc(blobB),
        )
        in_maps.append(m)
    return in_maps, W, MINACT


def assemble(core_outs):
    o = np.stack([np.asarray(co, np.float32) for co in core_outs])
    return np.ascontiguousarray(o.reshape(B * S, 256))


_CACHE = {}


def kernel(**inputs) -> np.ndarray:
    in_maps, W, MINACT = prep_in_maps(inputs)
    key = (tuple(W), tuple(MINACT))
    if _CACHE.get("key") != key:
        _CACHE["nc"] = _build(W, MINACT)
        _CACHE["key"] = key
    nc = _CACHE["nc"]
    res = run_bass_kernel_spmd(nc, in_maps, core_ids=list(range(NCORES)))
    return assemble([r["out"] for r in res.results])
